# revision 9
# baseline (speedup 1.0000x reference)
"""Trainium2 Bass kernel for a 2-layer MGU RNN (B=64, T=1024, I=128, H=256).

Strategy
--------
- Data-parallel over batch: 8 cores x 8 sequences each. Weights replicated.
- Each MGU layer's pre-activation is split into an input part (x @ Wx, computed
  in bulk per window of W timesteps, fully parallel) and a recurrent part
  (h @ Wh, inherently serial over T).
- Everything lives in a TRANSPOSED layout: hidden/gate dims on SBUF partitions,
  batch (8) on the free axis.  The per-step recurrent matmul is
  gates_T[128m, 8] += Wh_chunk[128k,128m].T @ h_T[128k, 8] (8 chunk matmuls),
  accumulated in PSUM on top of the window-precomputed input part + bias.
- tanh(x) = 2*sigmoid(2x) - 1 with the c-gate weights pre-doubled on the host,
  so the scalar engine only ever needs the Sigmoid table (one ACT op per step
  over all four 128-row gate chunks; table switches would cost 1.3us each).
- Offset state H = h + 1 (bias corrected by -colsum(Wh), and for layer 2 also
  -colsum(Wx2)) makes the gating arithmetic exactly 3 DVE ops per step:
      d = H_prev - 2*s_c            (= h - c)
      e = f * d
      H_new = 2*s_c + e             (= h_new + 1)
- Layer 2 consumes layer 1's transposed state window directly (bias correction
  absorbs the -1); only the y2 output subtracts 1, once per window.
- x (2MB bf16) and the full y output (8MB fp32) are SBUF-resident, so the
  kernel has exactly three input DMAs at the start and one output DMA at the
  end.  This matters doubly on trn2: hardware-decoded instructions (PE
  matmuls, DMA descriptors) have a single sync-wait slot, so the program is
  arranged so no such instruction ever needs two fresh semaphore waits
  (observer matmul for the weight DMA, bias-matmul first in each window).
- Outputs are written in transposed layout and fixed up on the host (pure
  layout transform).
"""

import os
from contextlib import ExitStack

import numpy as np

B_TOT, T_FULL, I_DIM, H_DIM = 64, 1024, 128, 256
N_CORES = 8
B_LOC = B_TOT // N_CORES  # 8

_T = int(os.environ.get("MGU_T", T_FULL))
_W = int(os.environ.get("MGU_W", 16))
_CAST = bool(int(os.environ.get("MGU_CAST", "1")))  # bf16 matmul operands
_WDT = os.environ.get("MGU_WDT", "bf16")  # weight dtype: bf16 | f32

_NC_CACHE = {}


def _prep_weights(Wf1, bf1, Wc1, bc1, Wf2, bf2, Wc2, bc2):
    """Host-side constant preprocessing (layout + algebraic folds only)."""

    def prep(Wf, bf, Wc, bc, in_dim):
        Wall = np.concatenate(
            [np.asarray(Wf, np.float32), 2.0 * np.asarray(Wc, np.float32)], axis=1
        )  # [in+H, 512]
        ball = np.concatenate(
            [np.asarray(bf, np.float32), 2.0 * np.asarray(bc, np.float32)]
        )  # [512]
        return Wall[:in_dim], Wall[in_dim:], ball

    Wx1, Wh1, b1 = prep(Wf1, bf1, Wc1, bc1, I_DIM)
    Wx2, Wh2, b2 = prep(Wf2, bf2, Wc2, bc2, H_DIM)
    # offset-state correction: matmul rhs carries H = h+1
    b1 = b1 - Wh1.sum(axis=0)
    b2 = b2 - Wh2.sum(axis=0) - Wx2.sum(axis=0)

    def chunk(W_):
        # [K, 512] -> [128(Krow), K/128 * 4, 128(Mcol)]; chunk index = k*4+m
        K = W_.shape[0]
        return W_.reshape(K // 128, 128, 4, 128).transpose(1, 0, 2, 3).reshape(
            128, -1, 128
        )

    wb = np.concatenate(
        [chunk(Wx1), chunk(Wh1), chunk(Wx2), chunk(Wh2)], axis=1
    )  # [128, 4+8+8+8=28, 128]
    return wb, b1.reshape(4, 128), b2.reshape(4, 128)


def _build_nc(T, W, cast, wdt_name):
    import concourse.bacc as bacc
    import concourse.mybir as mybir
    from concourse.tile import TileContext

    dt = mybir.dt
    wdt = dt.float32 if wdt_name == "f32" else dt.bfloat16
    sdt = wdt if cast else dt.float32  # dtype of matmul rhs operands
    NW = T // W
    TB = B_LOC * W  # free size of one window (= 128 when W=16)

    nc = bacc.Bacc(target_bir_lowering=False)

    x_d = nc.dram_tensor("xc", [128, T, B_LOC], sdt, kind="ExternalInput")
    wb_d = nc.dram_tensor("wb", [128, 28, 128], wdt, kind="ExternalInput")
    cst_d = nc.dram_tensor("cst", [4, 256 + 4 * TB], dt.float32, kind="ExternalInput")
    # y output carries the T timesteps plus 2 trailing slots for the final
    # hidden states of the two layers.
    y_d = nc.dram_tensor(
        "y", [128, 2, T + 2, B_LOC], dt.float32, kind="ExternalOutput"
    )

    with TileContext(nc) as tc, ExitStack() as ctx:
        consts = ctx.enter_context(tc.tile_pool(name="consts", bufs=1))
        hf_pool = ctx.enter_context(tc.tile_pool(name="hf32", bufs=3))
        hb_pool = ctx.enter_context(tc.tile_pool(name="hbf", bufs=3)) if cast else None
        s_pool = ctx.enter_context(tc.tile_pool(name="sgate", bufs=4))
        de_pool = ctx.enter_context(tc.tile_pool(name="descratch", bufs=4))
        g1_pool = ctx.enter_context(tc.tile_pool(name="g1", bufs=2, space="PSUM"))
        g2_pool = ctx.enter_context(tc.tile_pool(name="g2", bufs=2, space="PSUM"))
        obs_pool = ctx.enter_context(tc.tile_pool(name="obs", bufs=1, space="PSUM"))

        # --- resident tensors ---
        wbt = consts.tile([128, 28, 128], wdt)
        xft = consts.tile([128, T, B_LOC], sdt)
        cst = consts.tile([4, 256 + 4 * TB], dt.float32)
        yft = consts.tile([128, 2, T + 2, B_LOC], dt.float32)
        nc.gpsimd.dma_start(out=wbt[:], in_=wb_d[:])
        nc.gpsimd.dma_start(out=xft[:], in_=x_d[:])
        nc.gpsimd.dma_start(out=cst[:], in_=cst_d[:])
        b_ap = [cst[:, 0:128], cst[:, 128:256]]
        eye = cst[:, 256 : 256 + 4 * TB]

        def wxa(l, k, m):  # input-part weight chunk
            return wbt[:, (0 if l == 0 else 12) + k * 4 + m, :]

        def wha(l, k, m):  # recurrent weight chunk
            return wbt[:, (4 if l == 0 else 20) + k * 4 + m, :]

        hinit_f = consts.tile([128, 2, B_LOC], dt.float32)
        nc.vector.memset(hinit_f[:], 1.0)  # H0 = h0 + 1 = 1
        if cast:
            hinit_b = consts.tile([128, 2, B_LOC], wdt)
            nc.vector.memset(hinit_b[:], 1.0)

        # PE "observer" matmul: makes the PE sequencer observe the weight-DMA
        # semaphore up front, so no later (single-wait-slot) matmul ever needs
        # two fresh semaphore waits at once.
        obs = obs_pool.tile([1, 1], dt.float32)
        nc.tensor.matmul(
            obs[:, :], wbt[:, 0, 0:1], wbt[:, 0, 0:1],
            start=True, stop=True, skip_group_check=True,
        )

        g_pool = [g1_pool, g2_pool]
        kc_x = [1, 2]  # K-chunks of the input-part matmul per layer

        # Per-layer rolling state windows:
        #   hfw[l]: fp32 master [128, 2, W, B]
        #   hbw[l]: bf16 matmul copy (cast mode)
        hfw = [None, None]
        hbw = [None, None]
        hfw_prev = [None, None]
        hbw_prev = [None, None]
        gtile = [None, None]

        sig = mybir.ActivationFunctionType.Sigmoid
        mult = mybir.AluOpType.mult
        add = mybir.AluOpType.add

        def mm_rhs(l, t):
            """state matmul operand ([128, 2, B]) for layer l, step t-1."""
            if t == 0:
                prev = hbw_prev[l] if cast else hfw_prev[l]
                if prev is None:
                    return (hinit_b if cast else hinit_f)[:]
                return prev[:, :, W - 1, :]
            cur = hbw[l] if cast else hfw[l]
            return cur[:, :, t - 1, :]

        def bulk(l, win):
            """Window input part + bias into a fresh psum tile.

            Bias matmul FIRST with start=True: it touches every byte of the
            single-bank psum tile (zero-region replace covers the window) and
            its single wait slot absorbs the WAR dep on the psum buffer.
            """
            g = g_pool[l].tile([128, 4, TB], dt.float32, name=f"g{l}")
            gtile[l] = g
            nc.tensor.matmul(
                g[:, :, :], b_ap[l], eye,
                start=True, stop=False, skip_group_check=True,
            )
            if l == 0:
                rhs_k = [xft[:, win * W : (win + 1) * W, :]]
            else:
                src = hbw[0] if cast else hfw[0]
                rhs_k = [src[:, 0, :, :], src[:, 1, :, :]]
            for m in range(4):
                for k, rhs in enumerate(rhs_k):
                    nc.tensor.matmul(
                        g[:, m, :], wxa(l, k, m), rhs,
                        start=False, stop=False, skip_group_check=True,
                    )

        def step(l, t):
            """One recurrent timestep of layer l at window position t."""
            g = gtile[l]
            rhs = mm_rhs(l, t)
            for m in range(4):
                for k in range(2):
                    nc.tensor.matmul(
                        g[:, m, t * B_LOC : (t + 1) * B_LOC],
                        wha(l, k, m),
                        rhs[:, k, :],
                        start=False, stop=(k == 1), skip_group_check=True,
                    )
            s = s_pool.tile(
                [128, 4, B_LOC], dt.float32, tag=f"s{l}", name=f"s{l}"
            )
            nc.scalar.activation(s[:], g[:, :, t * B_LOC : (t + 1) * B_LOC], sig)
            f_ap = s[:, 0:2, :]
            sc_ap = s[:, 2:4, :]
            if t == 0:
                hprev = (
                    hfw_prev[l][:, :, W - 1, :]
                    if hfw_prev[l] is not None
                    else hinit_f[:]
                )
            else:
                hprev = hfw[l][:, :, t - 1, :]
            d = de_pool.tile([128, 2, B_LOC], dt.float32, tag=f"d{l}", name=f"d{l}")
            nc.vector.scalar_tensor_tensor(d[:], sc_ap, -2.0, hprev, mult, add)
            e = de_pool.tile([128, 2, B_LOC], dt.float32, tag=f"e{l}", name=f"e{l}")
            nc.vector.tensor_mul(e[:], f_ap, d[:])
            nc.vector.scalar_tensor_tensor(
                hfw[l][:, :, t, :], sc_ap, 2.0, e[:], mult, add
            )
            if cast:
                nc.vector.tensor_copy(hbw[l][:, :, t, :], hfw[l][:, :, t, :])

        def new_windows(l):
            hfw_prev[l] = hfw[l]
            hfw[l] = hf_pool.tile(
                [128, 2, W, B_LOC], dt.float32, tag=f"hf{l}", name=f"hf{l}"
            )
            if cast:
                hbw_prev[l] = hbw[l]
                hbw[l] = hb_pool.tile(
                    [128, 2, W, B_LOC], wdt, tag=f"hb{l}", name=f"hb{l}"
                )

        def ydump(win):
            # y2 = H2 - 1, staged into the resident output buffer
            nc.vector.tensor_scalar_add(
                yft[:, :, win * W : (win + 1) * W, :], hfw[1][:], -1.0
            )

        # --- main schedule ---
        for win in range(NW):
            l2_ready = win > 0
            if l2_ready:
                new_windows(1)
                bulk(1, win - 1)
            new_windows(0)
            bulk(0, win)
            for t in range(W):
                step(0, t)
                if l2_ready:
                    step(1, t)
            if l2_ready:
                ydump(win - 1)

        # tail: layer-2 window NW-1
        new_windows(1)
        bulk(1, NW - 1)
        for t in range(W):
            step(1, t)
        ydump(NW - 1)

        # final hidden states (H - 1) into the two trailing y slots
        nc.vector.tensor_scalar_add(yft[:, :, T, :], hfw[0][:, :, W - 1, :], -1.0)
        nc.vector.tensor_scalar_add(yft[:, :, T + 1, :], hfw[1][:, :, W - 1, :], -1.0)

        nc.gpsimd.dma_start(out=y_d[:], in_=yft[:])

    nc.finalize()
    return nc


def _get_nc():
    key = (_T, _W, _CAST, _WDT)
    if key not in _NC_CACHE:
        _NC_CACHE[key] = _build_nc(_T, _W, _CAST, _WDT)
    return _NC_CACHE[key]


def _make_in_maps(inputs):
    import ml_dtypes

    x = np.asarray(inputs["x"], np.float32)
    T = x.shape[1]
    assert T == _T, f"x has T={T} but kernel built for T={_T} (set MGU_T)"
    wb, b1, b2 = _prep_weights(
        inputs["Wf1"], inputs["bf1"], inputs["Wc1"], inputs["bc1"],
        inputs["Wf2"], inputs["bf2"], inputs["Wc2"], inputs["bc2"],
    )
    wdt = np.float32 if _WDT == "f32" else ml_dtypes.bfloat16
    sdt = wdt if _CAST else np.float32
    wb = np.ascontiguousarray(wb).astype(wdt)
    TB = B_LOC * _W
    eye = np.broadcast_to(
        np.eye(4, dtype=np.float32)[:, :, None], (4, 4, TB)
    ).reshape(4, 4 * TB)
    cst = np.ascontiguousarray(np.concatenate([b1, b2, eye], axis=1)).astype(
        np.float32
    )
    in_maps = []
    for c in range(N_CORES):
        xs = x[c * B_LOC : (c + 1) * B_LOC]  # [B, T, I]
        xt = np.ascontiguousarray(xs.transpose(2, 1, 0)).astype(sdt)  # [I, T, B]
        in_maps.append({"xc": xt, "wb": wb, "cst": cst})
    return in_maps


def _run(inputs, trace=False):
    from concourse.bass_utils import run_bass_kernel_spmd

    nc = _get_nc()
    in_maps = _make_in_maps(inputs)
    res = run_bass_kernel_spmd(
        nc, in_maps, core_ids=list(range(N_CORES)), trace=trace
    )
    T = _T
    y_parts = []
    h_parts = []
    for c in range(N_CORES):
        r = res.results[c]
        yt = r["y"]  # [128, 2, T+2, B]
        y_parts.append(
            np.transpose(yt[:, :, :T, :], (3, 2, 1, 0)).reshape(B_LOC, T, H_DIM)
        )
        ht = yt[:, :, T : T + 2, :]  # [128, 2, 2(layer), B]
        h_parts.append(np.transpose(ht, (2, 3, 1, 0)).reshape(2, B_LOC, H_DIM))
    y2 = np.concatenate(y_parts, axis=0)
    hidden = np.concatenate(h_parts, axis=1)
    return (y2, hidden), res


def kernel(**inputs):
    (y2, hidden), _ = _run(inputs, trace=False)
    return y2, hidden


# revision 10
# speedup vs baseline: 1.0487x; 1.0487x over previous
"""Trainium2 Bass kernel for a 2-layer MGU RNN (B=64, T=1024, I=128, H=256).

Strategy
--------
- Data-parallel over batch: 8 cores x 8 sequences each. Weights replicated.
- Each MGU layer's pre-activation is split into an input part (x @ Wx, computed
  in bulk per window of W timesteps, fully parallel) and a recurrent part
  (h @ Wh, inherently serial over T).
- Everything lives in a TRANSPOSED layout: hidden/gate dims on SBUF partitions,
  batch (8) on the free axis.  The per-step recurrent matmul is
  gates_T[128m, 8] += Wh_chunk[128k,128m].T @ h_T[128k, 8] (8 chunk matmuls),
  accumulated in PSUM on top of the window-precomputed input part + bias.
- tanh(x) = 2*sigmoid(2x) - 1 with the c-gate weights pre-doubled on the host,
  so the scalar engine only ever needs the Sigmoid table (one ACT op per step
  over all four 128-row gate chunks; table switches would cost 1.3us each).
- Offset state H = h + 1 (bias corrected by -colsum(Wh), and for layer 2 also
  -colsum(Wx2)) makes the gating arithmetic exactly 3 DVE ops per step:
      d = H_prev - 2*s_c            (= h - c)
      e = f * d
      H_new = 2*s_c + e             (= h_new + 1)
- Layer 2 consumes layer 1's transposed state window directly (bias correction
  absorbs the -1); only the y2 output subtracts 1, once per window.
- x (2MB bf16) and the full y output (8MB fp32) are SBUF-resident, so the
  kernel has exactly three input DMAs at the start and one output DMA at the
  end.  This matters doubly on trn2: hardware-decoded instructions (PE
  matmuls, DMA descriptors) have a single sync-wait slot, so the program is
  arranged so no such instruction ever needs two fresh semaphore waits
  (observer matmul for the weight DMA, bias-matmul first in each window).
- Outputs are written in transposed layout and fixed up on the host (pure
  layout transform).
"""

import os
from contextlib import ExitStack

import numpy as np

B_TOT, T_FULL, I_DIM, H_DIM = 64, 1024, 128, 256
N_CORES = 8
B_LOC = B_TOT // N_CORES  # 8

_T = int(os.environ.get("MGU_T", T_FULL))
_W = int(os.environ.get("MGU_W", 16))
_CAST = True  # bf16 matmul copy in h-space (fp32 master in H-space)
_WDT = os.environ.get("MGU_WDT", "bf16")  # weight dtype: bf16 | f32

_NC_CACHE = {}


def _prep_weights(Wf1, bf1, Wc1, bc1, Wf2, bf2, Wc2, bc2):
    """Host-side constant preprocessing (layout + algebraic folds only)."""

    def prep(Wf, bf, Wc, bc, in_dim):
        Wall = np.concatenate(
            [np.asarray(Wf, np.float32), 2.0 * np.asarray(Wc, np.float32)], axis=1
        )  # [in+H, 512]
        ball = np.concatenate(
            [np.asarray(bf, np.float32), 2.0 * np.asarray(bc, np.float32)]
        )  # [512]
        return Wall[:in_dim], Wall[in_dim:], ball

    Wx1, Wh1, b1 = prep(Wf1, bf1, Wc1, bc1, I_DIM)
    Wx2, Wh2, b2 = prep(Wf2, bf2, Wc2, bc2, H_DIM)

    def chunk(W_):
        # [K, 512] -> [128(Krow), K/128 * 4, 128(Mcol)]; chunk index = k*4+m
        K = W_.shape[0]
        return W_.reshape(K // 128, 128, 4, 128).transpose(1, 0, 2, 3).reshape(
            128, -1, 128
        )

    wb = np.concatenate(
        [chunk(Wx1), chunk(Wh1), chunk(Wx2), chunk(Wh2)], axis=1
    )  # [128, 4+8+8+8=28, 128]
    return wb, b1.reshape(4, 128), b2.reshape(4, 128)


def _build_nc(T, W, cast, wdt_name):
    import concourse.bacc as bacc
    import concourse.mybir as mybir
    from concourse.tile import TileContext

    dt = mybir.dt
    wdt = dt.float32 if wdt_name == "f32" else dt.bfloat16
    sdt = wdt if cast else dt.float32  # dtype of matmul rhs operands
    NW = T // W
    TB = B_LOC * W  # free size of one window (= 128 when W=16)

    nc = bacc.Bacc(target_bir_lowering=False)

    x_d = nc.dram_tensor("xc", [128, T, B_LOC], sdt, kind="ExternalInput")
    wb_d = nc.dram_tensor("wb", [128, 28, 128], wdt, kind="ExternalInput")
    cst_d = nc.dram_tensor("cst", [4, 256 + 4 * TB], dt.float32, kind="ExternalInput")
    # y output carries the T timesteps plus 2 trailing slots for the final
    # hidden states of the two layers.
    y_d = nc.dram_tensor(
        "y", [128, 2, T + 2, B_LOC], dt.float32, kind="ExternalOutput"
    )

    with TileContext(nc) as tc, ExitStack() as ctx:
        consts = ctx.enter_context(tc.tile_pool(name="consts", bufs=1))
        hf_pool = ctx.enter_context(tc.tile_pool(name="hf32", bufs=3))
        hb_pool = ctx.enter_context(tc.tile_pool(name="hbf", bufs=3)) if cast else None
        s_pool = ctx.enter_context(tc.tile_pool(name="sgate", bufs=4))
        de_pool = ctx.enter_context(tc.tile_pool(name="descratch", bufs=4))
        g1_pool = ctx.enter_context(tc.tile_pool(name="g1", bufs=2, space="PSUM"))
        g2_pool = ctx.enter_context(tc.tile_pool(name="g2", bufs=2, space="PSUM"))
        obs_pool = ctx.enter_context(tc.tile_pool(name="obs", bufs=1, space="PSUM"))

        # --- resident tensors ---
        wbt = consts.tile([128, 28, 128], wdt)
        xft = consts.tile([128, T, B_LOC], sdt)
        cst = consts.tile([4, 256 + 4 * TB], dt.float32)
        yft = consts.tile([128, 2, T + 2, B_LOC], dt.float32)
        nc.gpsimd.dma_start(out=wbt[:], in_=wb_d[:])
        nc.gpsimd.dma_start(out=xft[:], in_=x_d[:])
        nc.gpsimd.dma_start(out=cst[:], in_=cst_d[:])
        b_ap = [cst[:, 0:128], cst[:, 128:256]]
        eye = cst[:, 256 : 256 + 4 * TB]

        def wxa(l, k, m):  # input-part weight chunk
            return wbt[:, (0 if l == 0 else 12) + k * 4 + m, :]

        def wha(l, k, m):  # recurrent weight chunk
            return wbt[:, (4 if l == 0 else 20) + k * 4 + m, :]

        hinit_f = consts.tile([128, 2, B_LOC], dt.float32)
        nc.vector.memset(hinit_f[:], 1.0)  # H0 = h0 + 1 = 1
        if cast:
            hinit_b = consts.tile([128, 2, B_LOC], wdt)
            nc.vector.memset(hinit_b[:], 0.0)  # h-space copy: h0 = 0

        # PE "observer" matmul: makes the PE sequencer observe the weight-DMA
        # semaphore up front, so no later (single-wait-slot) matmul ever needs
        # two fresh semaphore waits at once.
        obs = obs_pool.tile([1, 1], dt.float32)
        nc.tensor.matmul(
            obs[:, :], wbt[:, 0, 0:1], wbt[:, 0, 0:1],
            start=True, stop=True, skip_group_check=True,
        )

        g_pool = [g1_pool, g2_pool]
        kc_x = [1, 2]  # K-chunks of the input-part matmul per layer

        # Per-layer rolling state windows:
        #   hfw[l]: fp32 master [128, 2, W, B]
        #   hbw[l]: bf16 matmul copy (cast mode)
        hfw = [None, None]
        hbw = [None, None]
        hfw_prev = [None, None]
        hbw_prev = [None, None]
        gtile = [None, None]

        sig = mybir.ActivationFunctionType.Sigmoid
        mult = mybir.AluOpType.mult
        add = mybir.AluOpType.add

        def mm_rhs(l, t):
            """state matmul operand ([128, 2, B]) for layer l, step t-1."""
            if t == 0:
                prev = hbw_prev[l] if cast else hfw_prev[l]
                if prev is None:
                    return (hinit_b if cast else hinit_f)[:]
                return prev[:, :, W - 1, :]
            cur = hbw[l] if cast else hfw[l]
            return cur[:, :, t - 1, :]

        def bulk(l, win):
            """Window input part + bias into a fresh psum tile.

            Bias matmul FIRST with start=True: it touches every byte of the
            single-bank psum tile (zero-region replace covers the window) and
            its single wait slot absorbs the WAR dep on the psum buffer.
            """
            g = g_pool[l].tile([128, 4, TB], dt.float32, name=f"g{l}")
            gtile[l] = g
            nc.tensor.matmul(
                g[:, :, :], b_ap[l], eye,
                start=True, stop=False, skip_group_check=True,
            )
            if l == 0:
                rhs_k = [xft[:, win * W : (win + 1) * W, :]]
            else:
                src = hbw[0] if cast else hfw[0]
                rhs_k = [src[:, 0, :, :], src[:, 1, :, :]]
            for m in range(4):
                for k, rhs in enumerate(rhs_k):
                    nc.tensor.matmul(
                        g[:, m, :], wxa(l, k, m), rhs,
                        start=False, stop=False, skip_group_check=True,
                    )

        def step(l, t):
            """One recurrent timestep of layer l at window position t."""
            g = gtile[l]
            rhs = mm_rhs(l, t)
            for m in range(4):
                for k in range(2):
                    nc.tensor.matmul(
                        g[:, m, t * B_LOC : (t + 1) * B_LOC],
                        wha(l, k, m),
                        rhs[:, k, :],
                        start=False, stop=(k == 1), skip_group_check=True,
                    )
            s = s_pool.tile(
                [128, 4, B_LOC], dt.float32, tag=f"s{l}", name=f"s{l}"
            )
            nc.scalar.activation(s[:], g[:, :, t * B_LOC : (t + 1) * B_LOC], sig)
            f_ap = s[:, 0:2, :]
            sc_ap = s[:, 2:4, :]
            if t == 0:
                hprev = (
                    hfw_prev[l][:, :, W - 1, :]
                    if hfw_prev[l] is not None
                    else hinit_f[:]
                )
            else:
                hprev = hfw[l][:, :, t - 1, :]
            d = de_pool.tile([128, 2, B_LOC], dt.float32, tag=f"d{l}", name=f"d{l}")
            nc.vector.scalar_tensor_tensor(d[:], sc_ap, -2.0, hprev, mult, add)
            e = de_pool.tile([128, 2, B_LOC], dt.float32, tag=f"e{l}", name=f"e{l}")
            nc.vector.tensor_mul(e[:], f_ap, d[:])
            nc.vector.scalar_tensor_tensor(
                hfw[l][:, :, t, :], sc_ap, 2.0, e[:], mult, add
            )
            if cast:
                # bf16 matmul copy holds h = H - 1 (quantizes around 0, not 1)
                nc.vector.tensor_scalar_add(
                    hbw[l][:, :, t, :], hfw[l][:, :, t, :], -1.0
                )

        def new_windows(l):
            hfw_prev[l] = hfw[l]
            hfw[l] = hf_pool.tile(
                [128, 2, W, B_LOC], dt.float32, tag=f"hf{l}", name=f"hf{l}"
            )
            if cast:
                hbw_prev[l] = hbw[l]
                hbw[l] = hb_pool.tile(
                    [128, 2, W, B_LOC], wdt, tag=f"hb{l}", name=f"hb{l}"
                )

        def ydump(win):
            # y2 = H2 - 1, staged into the resident output buffer
            nc.vector.tensor_scalar_add(
                yft[:, :, win * W : (win + 1) * W, :], hfw[1][:], -1.0
            )

        # --- main schedule ---
        for win in range(NW):
            l2_ready = win > 0
            if l2_ready:
                new_windows(1)
                bulk(1, win - 1)
            new_windows(0)
            bulk(0, win)
            for t in range(W):
                step(0, t)
                if l2_ready:
                    step(1, t)
            if l2_ready:
                ydump(win - 1)

        # tail: layer-2 window NW-1
        new_windows(1)
        bulk(1, NW - 1)
        for t in range(W):
            step(1, t)
        ydump(NW - 1)

        # final hidden states (H - 1) into the two trailing y slots
        nc.vector.tensor_scalar_add(yft[:, :, T, :], hfw[0][:, :, W - 1, :], -1.0)
        nc.vector.tensor_scalar_add(yft[:, :, T + 1, :], hfw[1][:, :, W - 1, :], -1.0)

        nc.gpsimd.dma_start(out=y_d[:], in_=yft[:])

    nc.finalize()
    return nc


def _get_nc():
    key = (_T, _W, _CAST, _WDT)
    if key not in _NC_CACHE:
        _NC_CACHE[key] = _build_nc(_T, _W, _CAST, _WDT)
    return _NC_CACHE[key]


def _make_in_maps(inputs):
    import ml_dtypes

    x = np.asarray(inputs["x"], np.float32)
    T = x.shape[1]
    assert T == _T, f"x has T={T} but kernel built for T={_T} (set MGU_T)"
    wb, b1, b2 = _prep_weights(
        inputs["Wf1"], inputs["bf1"], inputs["Wc1"], inputs["bc1"],
        inputs["Wf2"], inputs["bf2"], inputs["Wc2"], inputs["bc2"],
    )
    wdt = np.float32 if _WDT == "f32" else ml_dtypes.bfloat16
    sdt = wdt if _CAST else np.float32
    wb = np.ascontiguousarray(wb).astype(wdt)
    TB = B_LOC * _W
    eye = np.broadcast_to(
        np.eye(4, dtype=np.float32)[:, :, None], (4, 4, TB)
    ).reshape(4, 4 * TB)
    cst = np.ascontiguousarray(np.concatenate([b1, b2, eye], axis=1)).astype(
        np.float32
    )
    in_maps = []
    for c in range(N_CORES):
        xs = x[c * B_LOC : (c + 1) * B_LOC]  # [B, T, I]
        xt = np.ascontiguousarray(xs.transpose(2, 1, 0)).astype(sdt)  # [I, T, B]
        in_maps.append({"xc": xt, "wb": wb, "cst": cst})
    return in_maps


def _run(inputs, trace=False):
    from concourse.bass_utils import run_bass_kernel_spmd

    nc = _get_nc()
    in_maps = _make_in_maps(inputs)
    res = run_bass_kernel_spmd(
        nc, in_maps, core_ids=list(range(N_CORES)), trace=trace
    )
    T = _T
    y_parts = []
    h_parts = []
    for c in range(N_CORES):
        r = res.results[c]
        yt = r["y"]  # [128, 2, T+2, B]
        y_parts.append(
            np.transpose(yt[:, :, :T, :], (3, 2, 1, 0)).reshape(B_LOC, T, H_DIM)
        )
        ht = yt[:, :, T : T + 2, :]  # [128, 2, 2(layer), B]
        h_parts.append(np.transpose(ht, (2, 3, 1, 0)).reshape(2, B_LOC, H_DIM))
    y2 = np.concatenate(y_parts, axis=0)
    hidden = np.concatenate(h_parts, axis=1)
    return (y2, hidden), res


def kernel(**inputs):
    (y2, hidden), _ = _run(inputs, trace=False)
    return y2, hidden
